# revision 27
# baseline (speedup 1.0000x reference)
"""Trainium2 Bass kernel for AttnBlock (GroupNorm + 1x1-conv QKV + NxN attention
+ 1x1-conv proj + residual), data-parallel over batch across 8 NeuronCores.

Reference math (per batch b):
    h  = group_norm(x, gn_scale, gn_bias)          # 32 groups over [C, N]
    q/k/v = w @ h + b                              # [C, N] each, C=512
    S[i,j] = sum_c q[c,i] k[c,j] / sqrt(C)
    P  = softmax_j(S)
    o[c,i] = sum_j v[c,j] P[i,j]
    out = x + wp @ o + bp

Kernel strategy (one batch per core, everything resident in SBUF):
  - GN stats: per-partition bn_stats, cross-partition group mixing via a
    block-diagonal averaging matmul, rsqrt as exp(-0.5*ln(var+eps)).
  - Scores computed transposed: S^T[j,i] = k_tile^T q (stationary k-tile,
    moving q) in float32r; exp on ScalarE with the 1/sqrt(C) scale folded in,
    stored bf16 as expS [j, i].
  - Softmax denominators: elementwise accumulation of expS tiles on VectorE,
    then a ones-column matmul for the cross-partition reduce; reciprocal;
    partition-broadcast DMA.
  - PV: psum[c,i] accumulated over j-tiles with stationary vT[j,c] (bf16) and
    moving expS (bf16); normalized by rs broadcast on copy-back.
  - Proj conv reads the normalized o, residual x is re-read from DRAM and
    fused with the bias add via scalar_tensor_tensor.
  - The attention part (scores..proj) runs in two query-halves of 1024 to
    halve the SBUF footprint of expS/o; q/k/vT persist across halves.
All conv weights are pre-transposed on the host (wT[cin, cout]).
"""

import sys

sys.path.insert(0, "/opt/trn_rl_repo")

import numpy as np

import concourse.bacc as bacc
import concourse.bass as bass
import concourse.tile as tile
from concourse import mybir
from concourse.bass_utils import run_bass_kernel_spmd

AF = mybir.ActivationFunctionType
AXL = mybir.AxisListType
ALU = mybir.AluOpType
F32 = mybir.dt.float32
F32R = mybir.dt.float32r
BF16 = mybir.dt.bfloat16

C = 512  # channels
N = 2048  # sequence length
G = 32  # groups
EPS = 1e-6
P = 128  # partitions
T = C // P  # 4 channel tiles
NJ = N // P  # 16 key tiles
NH = N // 2  # query-half size (1024)
H5 = NH // 512  # 512-chunks per half (2)
SCALE = 1.0 / float(np.sqrt(C))
N_CORES = 8


def r32(ap):
    return ap.bitcast(F32R)


def build_program():
    nc = bacc.Bacc()

    x_d = nc.declare_dram_parameter("x", [C, N], F32, isOutput=False)
    xbf_d = nc.declare_dram_parameter("xbf", [C, N], BF16, isOutput=False)
    wqT_d = nc.declare_dram_parameter("wqT", [C, C], BF16, isOutput=False)
    wkT_d = nc.declare_dram_parameter("wkT", [C, C], BF16, isOutput=False)
    wvT_d = nc.declare_dram_parameter("wvT", [C, C], BF16, isOutput=False)
    wpT_d = nc.declare_dram_parameter("wpT", [C, C], BF16, isOutput=False)
    bq_d = nc.declare_dram_parameter("bq", [C], F32, isOutput=False)
    bk_d = nc.declare_dram_parameter("bk", [C], F32, isOutput=False)
    bv_d = nc.declare_dram_parameter("bv", [C], F32, isOutput=False)
    bp_d = nc.declare_dram_parameter("bp", [C], F32, isOutput=False)
    gns_d = nc.declare_dram_parameter("gns", [C], F32, isOutput=False)
    gnb_d = nc.declare_dram_parameter("gnb", [C], F32, isOutput=False)
    gblk_d = nc.declare_dram_parameter("gblk", [P, P], F32R, isOutput=False)
    sm_d = nc.declare_dram_parameter("smalls", [P, 20], F32, isOutput=False)
    out_d = nc.declare_dram_parameter("out", [C, N], F32, isOutput=True)

    with tile.TileContext(nc) as tc:
        build_tile_kernel(
            tc, x_d, xbf_d, wqT_d, wkT_d, wvT_d, wpT_d,
            bq_d, bk_d, bv_d, bp_d, gns_d, gnb_d, gblk_d, sm_d, out_d,
        )
    nc.finalize()
    return nc


def build_tile_kernel(tc, x_d, xbf_d, wqT_d, wkT_d, wvT_d, wpT_d,
                      bq_d, bk_d, bv_d, bp_d, gns_d, gnb_d, gblk_d, sm_d,
                      out_d):
    nc = tc.nc

    with (
        tc.tile_pool(name="const", bufs=1) as constp,
        tc.tile_pool(name="big", bufs=1) as bigp,
        tc.tile_pool(name="psA", bufs=5, space="PSUM") as psA,
        tc.tile_pool(name="psSC", bufs=3, space="PSUM") as psSC,
        tc.tile_pool(name="work", bufs=3) as workp,
    ):
        # ---- long-lived tensors -------------------------------------------
        vT_sb = bigp.tile([P, NJ, C], BF16, tag="vT")           # 16 KB/part
        expS_sb = bigp.tile([P, NJ, NH], BF16, tag="expS")      # 32 KB/part
        s_part = bigp.tile([P, NH], BF16, tag="spart")          # 2 KB/part

        sm_sb = constp.tile([P, 20], F32, tag="smalls")
        bq_sb = sm_sb[:, 0:4]
        bk_sb = sm_sb[:, 4:8]
        bp_sb = sm_sb[:, 8:12]
        ones_col = constp.tile([P, 1], BF16, tag="ones")
        nc.vector.memset(ones_col, 1.0)
        eps_col = constp.tile([P, 1], F32, tag="eps")
        nc.vector.memset(eps_col, EPS)

        with tc.tile_pool(name="qk", bufs=1) as qkp:
            q_sb = qkp.tile([P, T, N], BF16, tag="q")           # 16 KB/part
            k_sb = qkp.tile([P, T, N], BF16, tag="k")           # 16 KB/part

            with tc.tile_pool(name="xw", bufs=1) as xwp:
                x_sb = xwp.tile([P, T, N], BF16, tag="x")       # 16 KB/part
                h_sb = xwp.tile([P, T, N], BF16, tag="h")       # 16 KB/part
                wqT_sb = xwp.tile([P, T, C], BF16, tag="wqT")
                wkT_sb = xwp.tile([P, T, C], BF16, tag="wkT")
                wvT_sb = xwp.tile([P, T, C], BF16, tag="wvT")
                gblk_sb = xwp.tile([P, P], F32R, tag="gblk")
                bv_bc = xwp.tile([P, C], F32, tag="bvbc")
                gns_sb = sm_sb[:, 12:16]
                gnb_sb = sm_sb[:, 16:20]

                xr = xbf_d.rearrange("(t p) n -> p t n", p=P)
                for t in range(T):
                    for h in range(2):
                        nc.sync.dma_start(
                            out=x_sb[:, t, h * NH : (h + 1) * NH],
                            in_=xr[:, t, h * NH : (h + 1) * NH])
                nc.sync.dma_start(out=gblk_sb, in_=gblk_d.ap())
                nc.sync.dma_start(out=sm_sb, in_=sm_d.ap())
                wqr = wqT_d.rearrange("(t p) c -> p t c", p=P)
                wkr = wkT_d.rearrange("(t p) c -> p t c", p=P)
                wvr = wvT_d.rearrange("(t p) c -> p t c", p=P)
                for t in range(T):
                    nc.sync.dma_start(out=wqT_sb[:, t, :], in_=wqr[:, t, :])
                    nc.sync.dma_start(out=wkT_sb[:, t, :], in_=wkr[:, t, :])
                    nc.sync.dma_start(out=wvT_sb[:, t, :], in_=wvr[:, t, :])
                nc.sync.dma_start(
                    out=bv_bc, in_=bv_d.ap().partition_broadcast(P))

                # ---- PE warmup: dummy matmuls chase the x DMAs so the
                # HAM clock-gate is released before the conv burst -------
                for t in range(T):
                    for h in range(2):
                        ps = psA.tile([1, 512], F32, tag="mm")
                        nc.tensor.matmul(
                            ps, lhsT=x_sb[:, t, h * NH : h * NH + 1],
                            rhs=x_sb[:, t, h * NH : h * NH + 512],
                            start=True, stop=True)
                for w in range(10):
                    ps = psA.tile([1, 512], F32, tag="mm")
                    nc.tensor.matmul(
                        ps, lhsT=x_sb[:, T - 1, N - 1 : N],
                        rhs=x_sb[:, T - 1, N - 512 : N],
                        start=True, stop=True)

                # ---- GroupNorm ------------------------------------------
                # per-partition stats over the free dim (N), bn_stats in
                # 512-wide subchunks
                pstat = workp.tile([P, 2 * T], F32R, tag="pstat")
                for t in range(T):
                    st = workp.tile([P, N // 512, 6], F32, tag="bnst")
                    xv = x_sb[:, t, :].rearrange("p (s f) -> p s f", f=512)
                    for s in range(N // 512):
                        nc.vector.bn_stats(out=st[:, s, :], in_=xv[:, s, :])
                    mv = workp.tile([P, 2], F32, tag="bnmv")
                    nc.vector.bn_aggr(out=mv, in_=st)
                    # pstat[:, t] = mean;  pstat[:, T+t] = E[x^2]
                    nc.vector.tensor_copy(pstat[:, t : t + 1], mv[:, 0:1])
                    m2 = workp.tile([P, 1], F32, tag="bnm2")
                    nc.vector.tensor_mul(m2, mv[:, 0:1], mv[:, 0:1])
                    nc.vector.tensor_add(
                        pstat[:, T + t : T + t + 1], mv[:, 1:2], m2)

                # group mixing: gst[p, :] = mean over p's 16-block of pstat
                ps_g = psA.tile([P, 512], F32, tag="mm")
                nc.tensor.matmul(
                    ps_g[:, : 2 * T], lhsT=gblk_sb, rhs=pstat,
                    start=True, stop=True)
                gst = workp.tile([P, 2 * T], F32, tag="gst")
                nc.vector.tensor_copy(gst, ps_g[:, : 2 * T])

                gm = gst[:, 0:T]
                gvar = workp.tile([P, T], F32, tag="gvar")
                nc.vector.tensor_mul(gvar, gm, gm)
                nc.vector.tensor_sub(gvar, gst[:, T : 2 * T], gvar)
                # rstd = 1 / sqrt(var + eps)
                nc.scalar.activation(gvar, gvar, AF.Sqrt, bias=eps_col)
                nc.vector.reciprocal(gvar, gvar)
                # mscale = rstd * gn_scale ; madd = gn_bias - mean * mscale
                msc = workp.tile([P, T], F32, tag="msc")
                nc.vector.tensor_mul(msc, gvar, gns_sb)
                mad = workp.tile([P, T], F32, tag="mad")
                nc.vector.tensor_mul(mad, gm, msc)
                nc.vector.tensor_sub(mad, gnb_sb, mad)
                for hh in range(2):
                    for t in range(T):
                        lo = hh * NH
                        if t == 0:
                            nc.scalar.activation(
                                out=h_sb[:, t, lo : lo + NH],
                                in_=x_sb[:, t, lo : lo + NH], func=AF.Identity,
                                bias=mad[:, t : t + 1],
                                scale=msc[:, t : t + 1])
                        else:
                            nc.vector.tensor_scalar(
                                out=h_sb[:, t, lo : lo + NH],
                                in0=x_sb[:, t, lo : lo + NH],
                                scalar1=msc[:, t : t + 1],
                                scalar2=mad[:, t : t + 1],
                                op0=ALU.mult, op1=ALU.add)

                # ---- q, k convs -----------------------------------------
                for dst, wT, bias in ((q_sb, wqT_sb, bq_sb), (k_sb, wkT_sb, bk_sb)):
                    for cc in range(T):
                        for i5 in range(N // 512):
                            ps = psA.tile([P, 512], F32, tag="mm")
                            for tp in range(T):
                                nc.tensor.matmul(
                                    ps,
                                    lhsT=wT[:, tp, cc * P : (cc + 1) * P],
                                    rhs=h_sb[:, tp, i5 * 512 : (i5 + 1) * 512],
                                    start=(tp == 0), stop=(tp == T - 1))
                            nc.vector.tensor_scalar_add(
                                out=dst[:, cc, i5 * 512 : (i5 + 1) * 512],
                                in0=ps, scalar1=bias[:, cc : cc + 1])

                # ---- vT conv (stationary h-tiles, moving wvT) -----------
                for j in range(NJ):
                    ps = psA.tile([P, 512], F32, tag="mm")
                    for tp in range(T):
                        nc.tensor.matmul(
                            ps,
                            lhsT=h_sb[:, tp, j * P : (j + 1) * P],
                            rhs=wvT_sb[:, tp, :],
                            start=(tp == 0), stop=(tp == T - 1))
                    nc.vector.tensor_add(vT_sb[:, j, :], ps, bv_bc)

            # ---- attention in two query-halves --------------------------
            attnp = tc.alloc_tile_pool(name="attn", bufs=1)
            streamp = tc.alloc_tile_pool(name="stream", bufs=5)
            rs_row = attnp.tile([1, NH], F32, tag="rsrow")      # 4 KB/part
            rs_bc = attnp.tile([P, NH], F32, tag="rsbc")        # 4 KB/part
            o_sb = attnp.tile([P, T, NH], BF16, tag="o")        # 8 KB/part
            wpT_sb = attnp.tile([P, T, C], BF16, tag="wpT")      # 8 KB/part
            wpr = wpT_d.rearrange("(t p) c -> p t c", p=P)
            for t in range(T):
                nc.sync.dma_start(out=wpT_sb[:, t, :], in_=wpr[:, t, :])
            for ih in range(2):
                base = ih * NH

                # scores + exp + denominator accumulation
                for j in range(NJ):
                    for hh in range(H5):
                        ps = psSC.tile([P, 512], F32, tag="sc")
                        lo = base + hh * 512
                        for tp in range(T):
                            nc.tensor.matmul(
                                ps,
                                lhsT=k_sb[:, tp, j * P : (j + 1) * P],
                                rhs=q_sb[:, tp, lo : lo + 512],
                                start=(tp == 0), stop=(tp == T - 1))
                        hlo = hh * 512
                        nc.scalar.activation(
                            out=expS_sb[:, j, hlo : hlo + 512], in_=ps,
                            func=AF.Exp, scale=SCALE)
                        if j == 0:
                            nc.vector.tensor_copy(
                                s_part[:, hlo : hlo + 512],
                                expS_sb[:, j, hlo : hlo + 512])
                        else:
                            nc.vector.tensor_add(
                                s_part[:, hlo : hlo + 512],
                                s_part[:, hlo : hlo + 512],
                                expS_sb[:, j, hlo : hlo + 512])

                # denominator: partition reduce, broadcast, wide reciprocal
                for hh in range(H5):
                    ps = psA.tile([1, 512], F32, tag="mm")
                    nc.tensor.matmul(
                        ps, lhsT=ones_col,
                        rhs=s_part[:, hh * 512 : (hh + 1) * 512],
                        start=True, stop=True)
                    nc.scalar.activation(
                        out=rs_row[:, hh * 512 : (hh + 1) * 512], in_=ps,
                        func=AF.Identity)
                nc.gpsimd.partition_broadcast(rs_bc, rs_row[0:1, :])
                nc.vector.reciprocal(rs_bc, rs_bc)

                # PV + normalize, then proj + bias + residual, per i-chunk
                for hh in range(H5):
                    for cc in range(T):
                        ps = psA.tile([P, 512], F32, tag="mm")
                        for j in range(NJ):
                            nc.tensor.matmul(
                                ps,
                                lhsT=vT_sb[:, j, cc * P : (cc + 1) * P],
                                rhs=expS_sb[:, j, hh * 512 : (hh + 1) * 512],
                                start=(j == 0), stop=(j == NJ - 1))
                        nc.vector.tensor_mul(
                            o_sb[:, cc, hh * 512 : (hh + 1) * 512],
                            ps, rs_bc[:, hh * 512 : (hh + 1) * 512])

                    for c2 in range(T):
                        ps = psA.tile([P, 512], F32, tag="mm")
                        for cc in range(T):
                            nc.tensor.matmul(
                                ps,
                                lhsT=wpT_sb[:, cc, c2 * P : (c2 + 1) * P],
                                rhs=o_sb[:, cc, hh * 512 : (hh + 1) * 512],
                                start=(cc == 0), stop=(cc == T - 1))
                        lo = base + hh * 512
                        xt = streamp.tile([P, 512], F32, tag="xres")
                        nc.sync.dma_start(
                            out=xt,
                            in_=x_d.ap()[c2 * P : (c2 + 1) * P, lo : lo + 512])
                        ot = streamp.tile([P, 512], F32, tag="ot")
                        nc.vector.scalar_tensor_tensor(
                            out=ot, in0=ps, scalar=bp_sb[:, c2 : c2 + 1],
                            in1=xt, op0=ALU.add, op1=ALU.add)
                        nc.sync.dma_start(
                            out=out_d.ap()[c2 * P : (c2 + 1) * P, lo : lo + 512],
                            in_=ot)

            streamp.release()
            attnp.release()


_PROGRAM = None


def _get_program():
    global _PROGRAM
    if _PROGRAM is None:
        _PROGRAM = build_program()
    return _PROGRAM


def _make_in_maps(x, gn_scale, gn_bias, wq, bq, wk, bk, wv, bv, wp, bp):
    f = np.ascontiguousarray
    gblk = np.zeros((P, P), dtype=np.float32)
    for g in range(P // 16):
        gblk[g * 16 : (g + 1) * 16, g * 16 : (g + 1) * 16] = 1.0 / 16.0
    import ml_dtypes

    bf16 = ml_dtypes.bfloat16
    shared = {
        "wqT": f(np.asarray(wq, np.float32).T.astype(bf16)),
        "wkT": f(np.asarray(wk, np.float32).T.astype(bf16)),
        "wvT": f(np.asarray(wv, np.float32).T.astype(bf16)),
        "wpT": f(np.asarray(wp, np.float32).T.astype(bf16)),
        "bq": f(np.asarray(bq, np.float32)),
        "bk": f(np.asarray(bk, np.float32)),
        "bv": f(np.asarray(bv, np.float32)),
        "bp": f(np.asarray(bp, np.float32)),
        "gns": f(np.asarray(gn_scale, np.float32)),
        "gnb": f(np.asarray(gn_bias, np.float32)),
        "gblk": gblk,
        "smalls": np.stack(
            [np.asarray(a, np.float32).reshape(T, P).T for a in
             (bq, bk, bp, gn_scale, gn_bias)], axis=1,
        ).reshape(P, 20).copy(),
    }
    x = np.asarray(x, np.float32)
    return [
        dict(shared, x=f(x[b]), xbf=f(x[b].astype(bf16)))
        for b in range(N_CORES)
    ]


def run(inputs, trace=False):
    nc = _get_program()
    in_maps = _make_in_maps(**inputs)
    res = run_bass_kernel_spmd(
        nc, in_maps, core_ids=list(range(N_CORES)), trace=trace)
    out = np.stack([np.asarray(res.results[b]["out"]) for b in range(N_CORES)])
    return out.astype(np.float32), res


def kernel(**inputs):
    out, _ = run(inputs, trace=False)
    return out


# revision 29
# speedup vs baseline: 1.1811x; 1.1811x over previous
"""Trainium2 Bass kernel for AttnBlock (GroupNorm + 1x1-conv QKV + NxN attention
+ 1x1-conv proj + residual), data-parallel over batch across 8 NeuronCores.

Reference math (per batch b):
    h  = group_norm(x, gn_scale, gn_bias)          # 32 groups over [C, N]
    q/k/v = w @ h + b                              # [C, N] each, C=512
    S[i,j] = sum_c q[c,i] k[c,j] / sqrt(C)
    P  = softmax_j(S)
    o[c,i] = sum_j v[c,j] P[i,j]
    out = x + wp @ o + bp

Kernel strategy (one batch per core, everything resident in SBUF):
  - GN stats: per-partition bn_stats, cross-partition group mixing via a
    block-diagonal averaging matmul, rsqrt as exp(-0.5*ln(var+eps)).
  - Scores computed transposed: S^T[j,i] = k_tile^T q (stationary k-tile,
    moving q) in float32r; exp on ScalarE with the 1/sqrt(C) scale folded in,
    stored bf16 as expS [j, i].
  - Softmax denominators: elementwise accumulation of expS tiles on VectorE,
    then a ones-column matmul for the cross-partition reduce; reciprocal;
    partition-broadcast DMA.
  - PV: psum[c,i] accumulated over j-tiles with stationary vT[j,c] (bf16) and
    moving expS (bf16); normalized by rs broadcast on copy-back.
  - Proj conv reads the normalized o, residual x is re-read from DRAM and
    fused with the bias add via scalar_tensor_tensor.
  - The attention part (scores..proj) runs in two query-halves of 1024 to
    halve the SBUF footprint of expS/o; q/k/vT persist across halves.
All conv weights are pre-transposed on the host (wT[cin, cout]).
"""

import sys

sys.path.insert(0, "/opt/trn_rl_repo")

import numpy as np

import concourse.bacc as bacc
import concourse.bass as bass
import concourse.tile as tile
from concourse import mybir
from concourse.bass_utils import run_bass_kernel_spmd

AF = mybir.ActivationFunctionType
AXL = mybir.AxisListType
ALU = mybir.AluOpType
F32 = mybir.dt.float32
F32R = mybir.dt.float32r
BF16 = mybir.dt.bfloat16

C = 512  # channels
N = 2048  # sequence length
G = 32  # groups
EPS = 1e-6
P = 128  # partitions
T = C // P  # 4 channel tiles
NJ = N // P  # 16 key tiles
NH = N // 2  # query-half size (1024)
H5 = NH // 512  # 512-chunks per half (2)
SCALE = 1.0 / float(np.sqrt(C))
N_CORES = 8


def r32(ap):
    return ap.bitcast(F32R)


def build_program():
    nc = bacc.Bacc()

    x_d = nc.declare_dram_parameter("x", [C, N], F32, isOutput=False)
    xbf_d = nc.declare_dram_parameter("xbf", [C, N], BF16, isOutput=False)
    wqT_d = nc.declare_dram_parameter("wqT", [C, C], BF16, isOutput=False)
    wkT_d = nc.declare_dram_parameter("wkT", [C, C], BF16, isOutput=False)
    wvT_d = nc.declare_dram_parameter("wvT", [C, C], BF16, isOutput=False)
    wpT_d = nc.declare_dram_parameter("wpT", [C, C], BF16, isOutput=False)
    bq_d = nc.declare_dram_parameter("bq", [C], F32, isOutput=False)
    bk_d = nc.declare_dram_parameter("bk", [C], F32, isOutput=False)
    bv_d = nc.declare_dram_parameter("bv", [C], F32, isOutput=False)
    bp_d = nc.declare_dram_parameter("bp", [C], F32, isOutput=False)
    gns_d = nc.declare_dram_parameter("gns", [C], F32, isOutput=False)
    gnb_d = nc.declare_dram_parameter("gnb", [C], F32, isOutput=False)
    gblk_d = nc.declare_dram_parameter("gblk", [P, P], F32R, isOutput=False)
    sm_d = nc.declare_dram_parameter("smalls", [P, 20], F32, isOutput=False)
    out_d = nc.declare_dram_parameter("out", [C, N], F32, isOutput=True)

    with tile.TileContext(nc) as tc:
        build_tile_kernel(
            tc, x_d, xbf_d, wqT_d, wkT_d, wvT_d, wpT_d,
            bq_d, bk_d, bv_d, bp_d, gns_d, gnb_d, gblk_d, sm_d, out_d,
        )
    nc.finalize()
    return nc


def build_tile_kernel(tc, x_d, xbf_d, wqT_d, wkT_d, wvT_d, wpT_d,
                      bq_d, bk_d, bv_d, bp_d, gns_d, gnb_d, gblk_d, sm_d,
                      out_d):
    nc = tc.nc

    with (
        tc.tile_pool(name="const", bufs=1) as constp,
        tc.tile_pool(name="big", bufs=1) as bigp,
        tc.tile_pool(name="psA", bufs=4, space="PSUM") as psA,
        tc.tile_pool(name="psSC", bufs=3, space="PSUM") as psSC,
        tc.tile_pool(name="work", bufs=3) as workp,
    ):
        # ---- long-lived tensors -------------------------------------------
        vT_sb = bigp.tile([P, NJ, C], BF16, tag="vT")           # 16 KB/part
        expS_sb = bigp.tile([P, NJ, NH], BF16, tag="expS")      # 32 KB/part
        s_part = bigp.tile([P, NH], BF16, tag="spart")          # 2 KB/part

        sm_sb = constp.tile([P, 20], F32, tag="smalls")
        bq_sb = sm_sb[:, 0:4]
        bk_sb = sm_sb[:, 4:8]
        bp_sb = sm_sb[:, 8:12]
        ones_col = constp.tile([P, 1], BF16, tag="ones")
        nc.vector.memset(ones_col, 1.0)
        eps_col = constp.tile([P, 1], F32, tag="eps")
        nc.vector.memset(eps_col, EPS)

        with tc.tile_pool(name="qk", bufs=1) as qkp:
            q_sb = qkp.tile([P, T, N], BF16, tag="q")           # 16 KB/part
            k_sb = qkp.tile([P, T, N], BF16, tag="k")           # 16 KB/part

            with tc.tile_pool(name="xw", bufs=1) as xwp:
                x_sb = xwp.tile([P, T, N], BF16, tag="x")       # 16 KB/part
                h_sb = xwp.tile([P, T, N], BF16, tag="h")       # 16 KB/part
                wqT_sb = xwp.tile([P, T, C], BF16, tag="wqT")
                wkT_sb = xwp.tile([P, T, C], BF16, tag="wkT")
                wvT_sb = xwp.tile([P, T, C], BF16, tag="wvT")
                gblk_sb = xwp.tile([P, P], F32R, tag="gblk")
                bv_bc = xwp.tile([P, C], F32, tag="bvbc")
                gns_sb = sm_sb[:, 12:16]
                gnb_sb = sm_sb[:, 16:20]

                xr = xbf_d.rearrange("(t p) n -> p t n", p=P)
                for t in range(T):
                    for h in range(2):
                        nc.sync.dma_start(
                            out=x_sb[:, t, h * NH : (h + 1) * NH],
                            in_=xr[:, t, h * NH : (h + 1) * NH])
                nc.sync.dma_start(out=gblk_sb, in_=gblk_d.ap())
                nc.sync.dma_start(out=sm_sb, in_=sm_d.ap())
                wqr = wqT_d.rearrange("(t p) c -> p t c", p=P)
                wkr = wkT_d.rearrange("(t p) c -> p t c", p=P)
                wvr = wvT_d.rearrange("(t p) c -> p t c", p=P)
                for t in range(T):
                    nc.sync.dma_start(out=wqT_sb[:, t, :], in_=wqr[:, t, :])
                    nc.sync.dma_start(out=wkT_sb[:, t, :], in_=wkr[:, t, :])
                    nc.sync.dma_start(out=wvT_sb[:, t, :], in_=wvr[:, t, :])
                nc.sync.dma_start(
                    out=bv_bc, in_=bv_d.ap().partition_broadcast(P))

                # ---- PE warmup: dummy matmuls chase the x DMAs so the
                # HAM clock-gate is released before the conv burst -------
                for t in range(T):
                    for h in range(2):
                        ps = psA.tile([1, 512], F32, tag="mm")
                        nc.tensor.matmul(
                            ps, lhsT=x_sb[:, t, h * NH : h * NH + 1],
                            rhs=x_sb[:, t, h * NH : h * NH + 512],
                            start=True, stop=True)

                # ---- GroupNorm ------------------------------------------
                # per-partition stats over the free dim (N), bn_stats in
                # 512-wide subchunks
                pstat = workp.tile([P, 2 * T], F32R, tag="pstat")
                for t in range(T):
                    st = workp.tile([P, N // 512, 6], F32, tag="bnst")
                    xv = x_sb[:, t, :].rearrange("p (s f) -> p s f", f=512)
                    for s in range(N // 512):
                        nc.vector.bn_stats(out=st[:, s, :], in_=xv[:, s, :])
                    mv = workp.tile([P, 2], F32, tag="bnmv")
                    nc.vector.bn_aggr(out=mv, in_=st)
                    # pstat[:, t] = mean;  pstat[:, T+t] = E[x^2]
                    nc.vector.tensor_copy(pstat[:, t : t + 1], mv[:, 0:1])
                    m2 = workp.tile([P, 1], F32, tag="bnm2")
                    nc.vector.tensor_mul(m2, mv[:, 0:1], mv[:, 0:1])
                    nc.vector.tensor_add(
                        pstat[:, T + t : T + t + 1], mv[:, 1:2], m2)

                # group mixing: gst[p, :] = mean over p's 16-block of pstat
                ps_g = psA.tile([P, 512], F32, tag="mm")
                nc.tensor.matmul(
                    ps_g[:, : 2 * T], lhsT=gblk_sb, rhs=pstat,
                    start=True, stop=True)
                gst = workp.tile([P, 2 * T], F32, tag="gst")
                nc.vector.tensor_copy(gst, ps_g[:, : 2 * T])

                gm = gst[:, 0:T]
                gvar = workp.tile([P, T], F32, tag="gvar")
                nc.vector.tensor_mul(gvar, gm, gm)
                nc.vector.tensor_sub(gvar, gst[:, T : 2 * T], gvar)
                # rstd = 1 / sqrt(var + eps)
                nc.scalar.activation(gvar, gvar, AF.Sqrt, bias=eps_col)
                nc.vector.reciprocal(gvar, gvar)
                # mscale = rstd * gn_scale ; madd = gn_bias - mean * mscale
                msc = workp.tile([P, T], F32, tag="msc")
                nc.vector.tensor_mul(msc, gvar, gns_sb)
                mad = workp.tile([P, T], F32, tag="mad")
                nc.vector.tensor_mul(mad, gm, msc)
                nc.vector.tensor_sub(mad, gnb_sb, mad)
                for hh in range(2):
                    for t in range(T):
                        lo = hh * NH
                        if t == 0:
                            nc.scalar.activation(
                                out=h_sb[:, t, lo : lo + NH],
                                in_=x_sb[:, t, lo : lo + NH], func=AF.Identity,
                                bias=mad[:, t : t + 1],
                                scale=msc[:, t : t + 1])
                        else:
                            nc.vector.tensor_scalar(
                                out=h_sb[:, t, lo : lo + NH],
                                in0=x_sb[:, t, lo : lo + NH],
                                scalar1=msc[:, t : t + 1],
                                scalar2=mad[:, t : t + 1],
                                op0=ALU.mult, op1=ALU.add)

                for w in range(8):
                    ps = psA.tile([1, 512], F32, tag="mm")
                    nc.tensor.matmul(
                        ps, lhsT=h_sb[:, 1, 0:1], rhs=h_sb[:, 1, 0:512],
                        start=True, stop=True)

                # ---- q, k convs -----------------------------------------
                for dst, wT, bias in ((q_sb, wqT_sb, bq_sb), (k_sb, wkT_sb, bk_sb)):
                    for cc in range(T):
                        for i5 in range(N // 512):
                            ps = psA.tile([P, 512], F32, tag="mm")
                            for tp in range(T):
                                nc.tensor.matmul(
                                    ps,
                                    lhsT=wT[:, tp, cc * P : (cc + 1) * P],
                                    rhs=h_sb[:, tp, i5 * 512 : (i5 + 1) * 512],
                                    start=(tp == 0), stop=(tp == T - 1))
                            nc.vector.tensor_scalar_add(
                                out=dst[:, cc, i5 * 512 : (i5 + 1) * 512],
                                in0=ps, scalar1=bias[:, cc : cc + 1])

                # ---- vT conv (stationary h-tiles, moving wvT) -----------
                for j in range(NJ):
                    ps = psA.tile([P, 512], F32, tag="mm")
                    for tp in range(T):
                        nc.tensor.matmul(
                            ps,
                            lhsT=h_sb[:, tp, j * P : (j + 1) * P],
                            rhs=wvT_sb[:, tp, :],
                            start=(tp == 0), stop=(tp == T - 1))
                    nc.vector.tensor_add(vT_sb[:, j, :], ps, bv_bc)

            # ---- attention in two query-halves --------------------------
            attnp = tc.alloc_tile_pool(name="attn", bufs=1)
            streamp = tc.alloc_tile_pool(name="stream", bufs=4)
            rs_row = attnp.tile([1, NH], F32, tag="rsrow")      # 4 KB/part
            rs_bc = attnp.tile([P, NH], F32, tag="rsbc")        # 4 KB/part
            o_sb = attnp.tile([P, T, NH], BF16, tag="o")        # 8 KB/part
            wpT_sb = attnp.tile([P, T, C], BF16, tag="wpT")      # 8 KB/part
            wpr = wpT_d.rearrange("(t p) c -> p t c", p=P)
            for t in range(T):
                nc.sync.dma_start(out=wpT_sb[:, t, :], in_=wpr[:, t, :])
            for ih in range(2):
                base = ih * NH

                # scores + exp + denominator accumulation
                for j in range(NJ):
                    for hh in range(H5):
                        ps = psSC.tile([P, 512], F32, tag="sc")
                        lo = base + hh * 512
                        for tp in range(T):
                            nc.tensor.matmul(
                                ps,
                                lhsT=k_sb[:, tp, j * P : (j + 1) * P],
                                rhs=q_sb[:, tp, lo : lo + 512],
                                start=(tp == 0), stop=(tp == T - 1))
                        hlo = hh * 512
                        nc.scalar.activation(
                            out=expS_sb[:, j, hlo : hlo + 512], in_=ps,
                            func=AF.Exp, scale=SCALE)
                        if j == 0:
                            nc.vector.tensor_copy(
                                s_part[:, hlo : hlo + 512],
                                expS_sb[:, j, hlo : hlo + 512])
                        else:
                            nc.vector.tensor_add(
                                s_part[:, hlo : hlo + 512],
                                s_part[:, hlo : hlo + 512],
                                expS_sb[:, j, hlo : hlo + 512])

                # denominator: partition reduce, broadcast, wide reciprocal
                for hh in range(H5):
                    ps = psA.tile([1, 512], F32, tag="mm")
                    nc.tensor.matmul(
                        ps, lhsT=ones_col,
                        rhs=s_part[:, hh * 512 : (hh + 1) * 512],
                        start=True, stop=True)
                    nc.scalar.activation(
                        out=rs_row[:, hh * 512 : (hh + 1) * 512], in_=ps,
                        func=AF.Identity)
                nc.gpsimd.partition_broadcast(rs_bc, rs_row[0:1, :])
                nc.vector.reciprocal(rs_bc, rs_bc)

                # PV + normalize, then proj + bias + residual, per i-chunk
                for hh in range(H5):
                    for cc in range(T):
                        ps = psA.tile([P, 512], F32, tag="mm")
                        for j in range(NJ):
                            nc.tensor.matmul(
                                ps,
                                lhsT=vT_sb[:, j, cc * P : (cc + 1) * P],
                                rhs=expS_sb[:, j, hh * 512 : (hh + 1) * 512],
                                start=(j == 0), stop=(j == NJ - 1))
                        nc.vector.tensor_mul(
                            o_sb[:, cc, hh * 512 : (hh + 1) * 512],
                            ps, rs_bc[:, hh * 512 : (hh + 1) * 512])

                    for c2 in range(T):
                        ps = psA.tile([P, 512], F32, tag="mm")
                        for cc in range(T):
                            nc.tensor.matmul(
                                ps,
                                lhsT=wpT_sb[:, cc, c2 * P : (c2 + 1) * P],
                                rhs=o_sb[:, cc, hh * 512 : (hh + 1) * 512],
                                start=(cc == 0), stop=(cc == T - 1))
                        lo = base + hh * 512
                        xt = streamp.tile([P, 512], F32, tag="xres")
                        nc.sync.dma_start(
                            out=xt,
                            in_=x_d.ap()[c2 * P : (c2 + 1) * P, lo : lo + 512])
                        ot = streamp.tile([P, 512], F32, tag="ot")
                        nc.vector.scalar_tensor_tensor(
                            out=ot, in0=ps, scalar=bp_sb[:, c2 : c2 + 1],
                            in1=xt, op0=ALU.add, op1=ALU.add)
                        nc.sync.dma_start(
                            out=out_d.ap()[c2 * P : (c2 + 1) * P, lo : lo + 512],
                            in_=ot)

            streamp.release()
            attnp.release()


_PROGRAM = None


def _get_program():
    global _PROGRAM
    if _PROGRAM is None:
        _PROGRAM = build_program()
    return _PROGRAM


def _make_in_maps(x, gn_scale, gn_bias, wq, bq, wk, bk, wv, bv, wp, bp):
    f = np.ascontiguousarray
    gblk = np.zeros((P, P), dtype=np.float32)
    for g in range(P // 16):
        gblk[g * 16 : (g + 1) * 16, g * 16 : (g + 1) * 16] = 1.0 / 16.0
    import ml_dtypes

    bf16 = ml_dtypes.bfloat16
    shared = {
        "wqT": f(np.asarray(wq, np.float32).T.astype(bf16)),
        "wkT": f(np.asarray(wk, np.float32).T.astype(bf16)),
        "wvT": f(np.asarray(wv, np.float32).T.astype(bf16)),
        "wpT": f(np.asarray(wp, np.float32).T.astype(bf16)),
        "bq": f(np.asarray(bq, np.float32)),
        "bk": f(np.asarray(bk, np.float32)),
        "bv": f(np.asarray(bv, np.float32)),
        "bp": f(np.asarray(bp, np.float32)),
        "gns": f(np.asarray(gn_scale, np.float32)),
        "gnb": f(np.asarray(gn_bias, np.float32)),
        "gblk": gblk,
        "smalls": np.stack(
            [np.asarray(a, np.float32).reshape(T, P).T for a in
             (bq, bk, bp, gn_scale, gn_bias)], axis=1,
        ).reshape(P, 20).copy(),
    }
    x = np.asarray(x, np.float32)
    return [
        dict(shared, x=f(x[b]), xbf=f(x[b].astype(bf16)))
        for b in range(N_CORES)
    ]


def run(inputs, trace=False):
    nc = _get_program()
    in_maps = _make_in_maps(**inputs)
    res = run_bass_kernel_spmd(
        nc, in_maps, core_ids=list(range(N_CORES)), trace=trace)
    out = np.stack([np.asarray(res.results[b]["out"]) for b in range(N_CORES)])
    return out.astype(np.float32), res


def kernel(**inputs):
    out, _ = run(inputs, trace=False)
    return out
